# revision 5
# baseline (speedup 1.0000x reference)
"""LSEP loss kernel for Trainium2 (8 NeuronCores, data-parallel on batch).

loss = log1p( sum_b [ (sum_{c: t=0} e^{x_bc}) * (sum_{c: t=1} e^{-x_bc}) ] ) / B

Per-core layout: [128 partitions, K rows, 24 ch] tiles; each partition reads a
contiguous K*96B block of DRAM per tile. ACT computes e^x and e^-x (bf16 out),
GPSIMD builds the 0/1 masks from target, DVE applies masks (bf16 2x mode),
row-sums via a halving tree (24->12->6->reduce), and accumulates per-row
products of the two sums. Output: [128,1] per-core partial sums of
s_neg*s_pos; host sums partials and applies log1p (the gather step).
"""

import numpy as np

B = 2_000_000
C = 24
NCORES = 8
P = 128
K = 196
TILES = 10
RPC_RAW = B // NCORES            # 250_000 real rows per core
RPC = P * K * TILES              # 250_880 padded rows per core

HALVE = 2                        # halving-tree depth: 24 -> 12 -> 6 -> reduce

_cached = {}


def _build(rows, k, tiles, halve=HALVE):
    from contextlib import ExitStack

    import concourse.bacc as bacc
    import concourse.tile as tile
    from concourse import mybir

    f32 = mybir.dt.float32
    bf16 = mybir.dt.bfloat16
    i32 = mybir.dt.int32
    Alu = mybir.AluOpType
    Act = mybir.ActivationFunctionType
    X = mybir.AxisListType.X
    XY = mybir.AxisListType.XY

    nc = bacc.Bacc("TRN2", debug=False, num_devices=NCORES)
    x = nc.dram_tensor("x", [rows, C], f32, kind="ExternalInput").ap()
    t = nc.dram_tensor("t", [rows, C], i32, kind="ExternalInput").ap()
    out = nc.dram_tensor("o", [P, 1], f32, kind="ExternalOutput").ap()

    xv = x.rearrange("(i p k) c -> i p k c", p=P, k=k)
    tv = t.rearrange("(i p k) c -> i p k c", p=P, k=k)

    with tile.TileContext(nc) as tc, ExitStack() as ctx:
        io = ctx.enter_context(tc.tile_pool(name="io", bufs=2))
        ep = ctx.enter_context(tc.tile_pool(name="ep", bufs=2))
        mp = ctx.enter_context(tc.tile_pool(name="mp", bufs=2))
        hp = ctx.enter_context(tc.tile_pool(name="hp", bufs=2))
        sp = ctx.enter_context(tc.tile_pool(name="sp", bufs=2))
        accp = ctx.enter_context(tc.tile_pool(name="accp", bufs=1))
        acc = accp.tile([P, tiles, k], f32)  # per-tile row products

        def rowsum(src, name_tag):
            """Halving-tree reduce of src [P, k, C] bf16 -> [P, k] f32."""
            w = C
            cur = src
            lvl = 0
            while lvl < halve and w % 2 == 0:
                nxt = hp.tile([P, k, w // 2], bf16, tag=f"{name_tag}h{lvl}")
                nc.vector.tensor_add(nxt, cur[:, :, 0 : w // 2], cur[:, :, w // 2 : w])
                cur = nxt
                w //= 2
                lvl += 1
            red = sp.tile([P, k], f32, tag=f"{name_tag}r")
            nc.vector.tensor_reduce(out=red, in_=cur, axis=X, op=Alu.add)
            return red

        for i in range(tiles):
            xt = io.tile([P, k, C], f32, tag="x")
            tt = io.tile([P, k, C], i32, tag="t")
            nc.sync.dma_start(out=xt, in_=xv[i])
            nc.sync.dma_start(out=tt, in_=tv[i])
            e1 = ep.tile([P, k, C], bf16, tag="e1")
            e2 = ep.tile([P, k, C], bf16, tag="e2")
            nc.scalar.activation(out=e1, in_=xt, func=Act.Exp)              # e^x
            nc.scalar.activation(out=e2, in_=xt, func=Act.Exp, scale=-1.0)  # e^-x
            tf = mp.tile([P, k, C], bf16, tag="tf")
            nf = mp.tile([P, k, C], bf16, tag="nf")
            nc.gpsimd.tensor_copy(out=tf, in_=tt)                           # t
            nc.gpsimd.tensor_scalar(nf, tt, -1.0, 1.0, Alu.mult, Alu.add)   # 1-t
            nc.vector.tensor_mul(e1, e1, nf)   # neg terms: (1-t)*e^x
            nc.vector.tensor_mul(e2, e2, tf)   # pos terms: t*e^-x
            ns = rowsum(e1, "n")
            ps = rowsum(e2, "p")
            nc.vector.tensor_mul(acc[:, i, :], ns, ps)  # s_neg*s_pos per row
        a1 = accp.tile([P, 1], f32)
        nc.vector.tensor_reduce(out=a1, in_=acc, axis=XY, op=Alu.add)
        nc.sync.dma_start(out=out, in_=a1)
    nc.compile()
    return nc


def _get_nc():
    key = (RPC, K, TILES, HALVE)
    if key not in _cached:
        _cached[key] = _build(RPC, K, TILES)
    return _cached[key]


def _shard(input, target):
    in_maps = []
    for c in range(NCORES):
        xs = np.zeros((RPC, C), np.float32)
        ts = np.zeros((RPC, C), np.int32)
        xs[:RPC_RAW] = input[c * RPC_RAW : (c + 1) * RPC_RAW]
        ts[:RPC_RAW] = target[c * RPC_RAW : (c + 1) * RPC_RAW]
        in_maps.append({"x": xs, "t": ts})
    return in_maps


_last_results = None


def kernel(input, target):
    global _last_results
    input = np.ascontiguousarray(np.asarray(input, dtype=np.float32))
    target = np.ascontiguousarray(np.asarray(target, dtype=np.int32))
    assert input.shape == (B, C) and target.shape == (B, C)

    from concourse.bass_utils import run_bass_kernel_spmd

    nc = _get_nc()
    in_maps = _shard(input, target)
    res = run_bass_kernel_spmd(nc, in_maps, core_ids=list(range(NCORES)))
    _last_results = res
    total = float(np.sum([r["o"] for r in res.results], dtype=np.float64))
    return np.asarray(np.log1p(total) / B, dtype=np.float32)
